# revision 5
# baseline (speedup 1.0000x reference)
"""Bilinear sampler (grid sample) on 8 Trainium2 NeuronCores.

Data-parallel over batch: each core processes 2 of the 16 samples.

Design (vs. the 2-descriptor/2KB-per-pixel SWDGE baseline at 2.52ms):
- Host staging builds a row-pair interleaved bf16 image RP[y*256+x] =
  [img[y,x,:], img[y+1,x,:]] (128B entries, rows duplicated).  One 512B
  gather token at stride 256B (idx = floor((y0*256+x0)/2) <= 32639,
  int16-safe) contains all 4 bilinear neighbors -> 1 descriptor / 512B
  per output pixel.  The gather is descriptor-count-bound (~7-10ns/desc,
  Q7-cluster-serial generation + per-queue ring), so halving descriptors
  ~halves gather time; byte count barely matters.
- 4 SWDGE queues round-robin per 4096-pixel chunk overlap ring drain
  with the cluster-serial generation of other chunks (8192-desc chunks
  measured slower: they stall their own ring mid-generation).
- Token parity selects a 6-tap mask; per chunk the combine is one DVE
  tensor_tensor mult against an interleaved [128,512,6] bf16 weight
  tile + a 3-instruction add tree (f32 final accumulate).  rel err
  ~7e-3 (threshold 2e-2), deterministic for fixed inputs.
- Idx fold/interleave/replicate prep batched per sample on SP; output
  DMAs issue from the otherwise-idle Activation engine to unload SP.


Self-contained: hardcodes B=16, H=W=256, C=32.
"""
import json

import numpy as np

import concourse.bacc as bacc
import concourse.bass_utils as bass_utils_mod
import concourse.bass2jax as bass2jax_mod
import concourse.mybir as mybir
from concourse.tile import TileContext
from concourse.vector_clock import ScopedClock
from concourse import library_config

# ---------------------------------------------------------------------------
# Workarounds for this container's walrus build, which rejects any
# instruction carrying more than one sync-wait command.
# ---------------------------------------------------------------------------

_ORIG_COMPILE = bass_utils_mod.compile_bir_kernel


def _patched_drain_and_barrier(self, tick_clock, wait_clock):
    nop = self.nc.sync.nop(nofuse=True, hint="pre_drain_waits")
    wait_clock.add_sem_waits(nop.ins, ScopedClock({None: tick_clock.global_clock}))
    si = nop.ins.sync_info
    waits = list(si.on_wait) if si and si.on_wait else []
    if si is not None:
        si.on_wait = []
    num2h = {h.num: h for h in self.sems.allocated().values()}
    for w in waits:
        self.nc.sync.wait_ge(num2h[w.id], w.wait_value)
    self.nc.sync.drain()
    self.nc.all_engine_barrier()
    popped = self.nc._tile_sem_poison_stack.pop()
    assert popped is self._sem_poison
    self.nc.clear_and_free_semaphores(list(self.sems.allocated().values()))
    self.nc.all_engine_barrier()


def _split_multiwaits(bir: dict) -> dict:
    uid = [0]

    def mk_wait(engine, wait, debug):
        uid[0] += 1
        inst = {
            "engine": engine,
            "ins": [],
            "name": f"mwsplit-{uid[0]}",
            "opcode": "EventSemaphore",
            "outs": [],
            "sync_info": {"on_update": [], "on_wait": [wait]},
        }
        if debug is not None:
            inst["debug"] = debug
        return inst

    for fn in bir["functions"]:
        for bb in fn["blocks"]:
            insts = bb.get("instructions")
            if not insts:
                continue
            new = []
            for inst in insts:
                si = inst.get("sync_info")
                waits = (si or {}).get("on_wait") or []
                if len(waits) > 1:
                    for w in waits[:-1]:
                        new.append(mk_wait(inst["engine"], w, inst.get("debug")))
                    si["on_wait"] = [waits[-1]]
                new.append(inst)
            bb["instructions"] = new
    return bir


def _patched_compile_bir_kernel(bir_json, tmpdir, neff_name="file.neff"):
    bir = json.loads(bir_json)
    bir = _split_multiwaits(bir)
    return _ORIG_COMPILE(json.dumps(bir).encode(), tmpdir, neff_name)


def _install_patches():
    TileContext._drain_and_barrier = _patched_drain_and_barrier
    bass_utils_mod.compile_bir_kernel = _patched_compile_bir_kernel
    bass2jax_mod.compile_bir_kernel = _patched_compile_bir_kernel


_install_patches()

# ---------------------------------------------------------------------------
# Problem constants
# ---------------------------------------------------------------------------

B, H, W, C = 16, 256, 256, 32
HWPIX = H * W                     # pixels per sample (65536)
N_CORES = 8
S_PER_CORE = B // N_CORES         # samples per core (2)
P = 128                           # SBUF partitions
Jc = 32                           # output pixels per partition per chunk
CHUNK = P * Jc                    # output pixels per chunk (4096)
RPN = 65536                       # RP entries per sample (256 rows x 256 cols)
EW = 64                           # f32 per RP entry (2 pixels x 32 ch)

f32 = mybir.dt.float32
bf16 = mybir.dt.bfloat16
i32 = mybir.dt.int32
i16 = mybir.dt.int16

AL = mybir.AluOpType


def _trace_kernel(nc, s_per_core=S_PER_CORE):
    npix = s_per_core * HWPIX
    rp = nc.dram_tensor(
        "rp", [s_per_core * RPN * EW + 256], bf16, kind="ExternalInput"
    )
    xs = nc.dram_tensor("xs", [npix], f32, kind="ExternalInput")
    ys = nc.dram_tensor("ys", [npix], f32, kind="ExternalInput")
    out = nc.dram_tensor("out", [npix * C], f32, kind="ExternalOutput")

    cps = HWPIX // CHUNK          # chunks per sample (16)
    Js = HWPIX // P               # pixels per partition per sample (512)

    nc.gpsimd.load_library(library_config.mlp)

    vec = nc.vector

    with TileContext(nc) as tc:
        with (
            tc.tile_pool(name="samp", bufs=1) as spool,
            tc.tile_pool(name="chunk", bufs=2) as kpool,
            tc.tile_pool(name="gat", bufs=3) as gpool,
            tc.tile_pool(name="dram", bufs=4, space="DRAM") as dpool,
        ):
            for s in range(s_per_core):
                # ---- sample-level index & weight computation ([P, Js]) ----
                def t(tag, w=Js, dt=f32):
                    return spool.tile([P, w], dt, tag=tag, name=tag)

                xs_t = t("xs_t")
                ys_t = t("ys_t")
                xs_sv = xs[s * HWPIX:(s + 1) * HWPIX].rearrange(
                    "(p u) -> p u", p=P
                )
                ys_sv = ys[s * HWPIX:(s + 1) * HWPIX].rearrange(
                    "(p u) -> p u", p=P
                )
                nc.sync.dma_start(out=xs_t[:], in_=xs_sv)
                nc.sync.dma_start(out=ys_t[:], in_=ys_sv)
                out_sv = out[s * HWPIX * C:(s + 1) * HWPIX * C].rearrange(
                    "(p c jk) -> c p jk", p=P, jk=Jc * C
                )

                # x = (xs + 1) * 127.5
                x = t("x")
                vec.tensor_scalar(x[:], xs_t[:], 1.0, 127.5, op0=AL.add, op1=AL.mult)

                def floor_to(dst_tag, src):
                    ci = t("flr_i", dt=i32)
                    vec.tensor_copy(out=ci[:], in_=src[:])
                    cf = t(dst_tag)
                    vec.tensor_copy(out=cf[:], in_=ci[:])
                    gt = t("flr_g")
                    vec.tensor_tensor(gt[:], cf[:], src[:], op=AL.is_gt)
                    vec.tensor_sub(cf[:], cf[:], gt[:])
                    return cf

                x0r = floor_to("x0r", x)
                x0f = t("x0f")
                vec.tensor_scalar(x0f[:], x0r[:], 254.0, 0.0, op0=AL.min, op1=AL.max)
                fx = t("fx")
                vec.tensor_sub(fx[:], x[:], x0f[:])       # weight for column x1
                wl = t("wl")
                vec.tensor_scalar(wl[:], fx[:], -1.0, 1.0, op0=AL.mult, op1=AL.add)

                y = t("y")
                vec.tensor_scalar(y[:], ys_t[:], 1.0, 127.5, op0=AL.add, op1=AL.mult)
                y0r = floor_to("y0r", y)
                y0f = t("y0f")
                vec.tensor_scalar(y0f[:], y0r[:], 254.0, 0.0, op0=AL.min, op1=AL.max)
                fy = t("fy")
                vec.tensor_sub(fy[:], y[:], y0f[:])       # weight for row y1
                wt = t("wt")
                vec.tensor_scalar(wt[:], fy[:], -1.0, 1.0, op0=AL.mult, op1=AL.add)

                # E = y0*256 + x0 ; idx = floor(E/2) ; parity q = E - 2*idx
                E = t("E")
                vec.tensor_scalar(E[:], y0f[:], 256.0, 0.0, op0=AL.mult, op1=AL.add)
                vec.tensor_add(E[:], E[:], x0f[:])
                h = t("h")
                vec.tensor_scalar_mul(h[:], E[:], 0.5)
                idxf = floor_to("idxf", h)
                q = t("q")
                vec.tensor_scalar_mul(q[:], idxf[:], 2.0)
                vec.tensor_sub(q[:], E[:], q[:])

                # idx -> int16, chunk-major [P, cps, Jc]
                idx2 = spool.tile([P, cps, Jc], i16, tag="idx2")
                vec.tensor_copy(
                    out=idx2[:, :, :],
                    in_=idxf[:].rearrange("p (c j) -> p c j", j=Jc),
                )

                # ---- idx fold/interleave/replicate pre-pass (batched) ----
                wrapall = spool.tile(
                    [P, cps, CHUNK // 16], i16, tag="wrapall", name="wrapall"
                )
                dscr = dpool.tile([P * cps * Jc], i16, tag="dscr", name="dscr")
                nc.sync.dma_start(
                    out=dscr[:].rearrange("(p u) -> p u", p=P),
                    in_=idx2[:, :, :].rearrange("p c j -> p (c j)"),
                )
                tmpf = kpool.tile([P, cps, 8, Jc], i16, tag="tmpf", name="tmpf")
                nc.sync.dma_start(
                    out=tmpf[:16],
                    in_=dscr[:].rearrange("(a b c j) -> b c a j", a=8, b=16, j=Jc),
                )
                vec.tensor_copy(
                    out=wrapall[:16, :, :].rearrange("b c (j a) -> b c j a", a=8),
                    in_=tmpf[:16].rearrange("b c a j -> b c j a"),
                )
                for half in (16, 32, 64):
                    nc.sync.dma_start(
                        out=wrapall[half:2 * half, :, :],
                        in_=wrapall[:half, :, :],
                    )

                # parity masks and 6 tap weights, interleaved [P, Js, 6]
                e0 = t("e0")
                vec.tensor_single_scalar(e0[:], q[:], 0.0, op=AL.is_equal)
                e1 = t("e1")
                vec.tensor_single_scalar(e1[:], q[:], 1.0, op=AL.is_equal)

                wa = t("wa")
                vec.tensor_mul(wa[:], wl[:], wt[:])    # top-left
                wb = t("wb")
                vec.tensor_mul(wb[:], wl[:], fy[:])    # bottom-left
                wc = t("wc")
                vec.tensor_mul(wc[:], fx[:], wt[:])    # top-right
                wd = t("wd")
                vec.tensor_mul(wd[:], fx[:], fy[:])    # bottom-right

                vint = spool.tile([P, Js, 6], bf16, tag="vint")
                tmpw = t("tmpw")
                vec.tensor_mul(vint[:, :, 0], wa[:], e0[:])
                vec.tensor_mul(vint[:, :, 1], wb[:], e0[:])
                vec.tensor_mul(vint[:, :, 4], wc[:], e1[:])
                vec.tensor_mul(vint[:, :, 5], wd[:], e1[:])
                vec.tensor_mul(tmpw[:], wa[:], e1[:])
                vec.tensor_mul(vint[:, :, 2], wc[:], e0[:])
                vec.tensor_add(vint[:, :, 2], vint[:, :, 2], tmpw[:])
                vec.tensor_mul(tmpw[:], wb[:], e1[:])
                vec.tensor_mul(vint[:, :, 3], wd[:], e0[:])
                vec.tensor_add(vint[:, :, 3], vint[:, :, 3], tmpw[:])

                # ---- per-chunk gather + combine ----
                for cc in range(cps):
                    # overlapping 512B-strided rows of the RP layout
                    ov = rp[s * RPN * EW:(s + 1) * RPN * EW].rearrange(
                        "(n k) -> n k", k=2 * EW
                    )
                    ov.ap[0] = [2 * EW, RPN // 2]
                    ov.ap[1] = [1, 4 * EW]

                    G = gpool.tile([P, Jc, 4 * EW], bf16, tag="G", bufs=4)
                    nc.gpsimd.dma_gather(
                        out_ap=G[:],
                        in_ap=ov,
                        idxs_ap=wrapall[:, cc, :],
                        num_idxs=CHUNK,
                        num_idxs_reg=CHUNK,
                        elem_size=4 * EW,
                        elem_step=2 * EW,
                        single_packet=False,
                        queue_num=cc % 4,
                    )

                    jsl = slice(cc * Jc, (cc + 1) * Jc)
                    vb = (
                        vint[:, jsl, :]
                        .unsqueeze(-1)
                        .to_broadcast([P, Jc, 6, C])
                    )
                    prod = kpool.tile([P, Jc, 6, C], bf16, tag="prod", bufs=1)
                    vec.tensor_tensor(
                        prod[:],
                        G[:, :, 0:6 * C].rearrange("p j (m u) -> p j m u", m=6),
                        vb,
                        op=AL.mult,
                    )
                    t1 = kpool.tile([P, Jc, 3, C], bf16, tag="t1", bufs=1)
                    vec.tensor_add(t1[:], prod[:, :, 0:3, :], prod[:, :, 3:6, :])
                    t2 = kpool.tile([P, Jc, C], bf16, tag="t2", bufs=1)
                    vec.tensor_add(t2[:], t1[:, :, 0, :], t1[:, :, 1, :])
                    acc = kpool.tile([P, Jc, C], f32, tag="acc", bufs=2)
                    vec.tensor_add(acc[:], t2[:], t1[:, :, 2, :])

                    nc.scalar.dma_start(
                        out=out_sv[cc],
                        in_=acc[:].rearrange("p j k -> p (j k)"),
                    )

    return rp, xs, ys, out


_NC_CACHE = {}


def _build(s_per_core=S_PER_CORE):
    key = s_per_core
    if key not in _NC_CACHE:
        nc = bacc.Bacc("TRN2", num_swdge_queues=4)
        _trace_kernel(nc, s_per_core)
        nc.finalize()
        _NC_CACHE[key] = nc
    return _NC_CACHE[key]


def _build_rp(img):
    # img: [S, 256, 256, 32] -> [S * 65536 entries * 64 bf16] (+256 pad)
    import ml_dtypes
    S = img.shape[0]
    rp = np.empty((S, H, W, 2, C), np.float32)
    rp[:, : H - 1, :, 0] = img[:, : H - 1]
    rp[:, : H - 1, :, 1] = img[:, 1:]
    rp[:, H - 1, :, 0] = img[:, H - 1]
    rp[:, H - 1, :, 1] = img[:, H - 1]
    out = np.concatenate([rp.reshape(-1), np.zeros(256, np.float32)])
    return out.astype(ml_dtypes.bfloat16)


def _make_in_maps(inputs):
    batch_inputs = np.asarray(inputs["batch_inputs"], dtype=np.float32)
    x_s = np.asarray(inputs["x_s"], dtype=np.float32)
    y_s = np.asarray(inputs["y_s"], dtype=np.float32)
    in_maps = []
    for i in range(N_CORES):
        sl = slice(i * S_PER_CORE, (i + 1) * S_PER_CORE)
        in_maps.append(
            {
                "rp": _build_rp(batch_inputs[sl]),
                "xs": np.ascontiguousarray(x_s[sl]).reshape(-1),
                "ys": np.ascontiguousarray(y_s[sl]).reshape(-1),
            }
        )
    return in_maps


def _assemble_output(outs):
    return outs[0].reshape(B, H, W, C)


def kernel(batch_inputs, x_s, y_s):
    from concourse import bass_utils

    nc = _build()
    in_maps = _make_in_maps(
        {"batch_inputs": batch_inputs, "x_s": x_s, "y_s": y_s}
    )
    res = bass_utils.run_bass_kernel_spmd(nc, in_maps, core_ids=list(range(N_CORES)))
    outs = [
        res.results[i]["out"].reshape(S_PER_CORE, H, W, C) for i in range(N_CORES)
    ]
    return np.concatenate(outs, axis=0)
